# revision 19
# baseline (speedup 1.0000x reference)
"""Trainium2 Bass kernel for nn_AttentionPromptExtrapolation.

Reference computation (B,N,P,D,K = 32,512,25,128,64):
    keep[n,p] = (n not in s_mti) and (p != 24)            # {0,1}, same for all b
    su = sigmoid(patches @ u.T);  su *= (su>0.5) * keep
    sm = sigmoid(patches @ m.T);  sm *= (sm>0.5) * (1-keep)
    out = patches + su @ u + sm @ m

Key observation: each row (b,n,p) uses exactly ONE of the two prompt tables
(u if keep, m otherwise). The host permutes rows so all keep-rows come
first, padded to 512-row sub-block boundaries (102 sub-blocks per core).
Each sub-block is then "pure" and needs a single K=64 score matmul against
its table — no masking on the device at all:

    z  = x_block @ T.T          [64, 512]  (T = u or m by group)
    st = (z > 0) * sigmoid(z)   (sigmoid(z) > 0.5  <=>  z > 0)
    out_block = x_block + st.T @ T

Layout: the host ships patches TRANSPOSED ([D, rows] row-major) so the
contraction dim D sits on SBUF partitions with contiguous 12KB-per-partition
DMA chunks — no on-chip transposes. The whole pipeline runs in transposed
space; the host un-permutes/un-transposes the result. Two sub-blocks' scores
are packed into the two partition halves of one PSUM bank (tile_position
column offset) so sigmoid and the threshold op each run once per PAIR.
The second matmul / add / store stage runs one pair behind the score stage
so the PE never stalls waiting on sigmoid/STT.
"""

import numpy as np

import concourse.bacc as bacc
import concourse.tile as tile
from concourse import mybir
from concourse.alu_op_type import AluOpType

B, N, P, D, K = 32, 512, 25, 128, 64
K2 = 2 * K              # 128
NCORES = 8
BPC = B // NCORES       # batches per core = 4
NP = N * P              # rows per batch = 12800
BLK = 512               # rows per compute sub-block
NBLOCKS = 102           # sub-blocks per core after group padding (51 pairs)
MB = 6                  # sub-blocks per megablock DMA (17 megablocks)
MBROWS = MB * BLK       # 3072 rows = 1.5 MB per DMA
ROWS = NBLOCKS * BLK    # 52224 padded rows per core
T_MTI = 24

F32 = mybir.dt.float32
F16 = mybir.dt.float16


def build_nc(nk):
    """Build the single-core bass program. Sub-blocks [0, nk) are keep-group
    (use table u = C[0:64]); [nk, NBLOCKS) are masked-group (m = C[64:128])."""
    nc = bacc.Bacc(None, target_bir_lowering=False)

    x_d = nc.dram_tensor("x", [D, ROWS], F32, kind="ExternalInput")       # x.T
    ct_d = nc.dram_tensor("ct", [D, K2], F32, kind="ExternalInput")       # C.T
    # C fp16 replicated in both partition halves: [u | m] on parts 0:64
    # and again on parts 64:128 (mm2's contraction partitions must match
    # whichever half of st it consumes)
    cb_d = nc.dram_tensor("cboth", [K2, 2 * D], F16, kind="ExternalInput")
    out_d = nc.dram_tensor("out", [D, ROWS], F32, kind="ExternalOutput")  # out.T

    def grp(s):
        return 0 if s < nk else 1

    with tile.TileContext(nc) as tc:
        with (
            tc.tile_pool(name="consts", bufs=1) as consts,
            tc.tile_pool(name="xp", bufs=4) as xp,
            tc.tile_pool(name="sgp", bufs=3) as sgp,
            tc.tile_pool(name="stp", bufs=3) as stp,
            tc.tile_pool(name="op", bufs=3) as op,
            tc.tile_pool(name="ps_z", bufs=3, space="PSUM") as ps_z,
            tc.tile_pool(name="ps_y", bufs=4, space="PSUM") as ps_y,
        ):
            ct_sb = consts.tile([D, K2], F32)
            nc.sync.dma_start(ct_sb, ct_d[:, :])
            cb_sb = consts.tile([K2, 2 * D], F16)
            nc.sync.dma_start(cb_sb, cb_d[:, :])

            x_mb = o_mb = None
            pend = None  # previous pair's (st_sb, x_mb, o_mb, pair_idx)

            def flush(pend):
                st_sb, px_mb, po_mb, pp = pend
                for slot in range(2):
                    s = 2 * pp + slot
                    g = grp(s)
                    msub = s % MB
                    y_ps = ps_y.tile([128, BLK], F32)
                    nc.tensor.matmul(
                        y_ps,
                        lhsT=cb_sb[slot * K:(slot + 1) * K,
                                   g * D:(g + 1) * D],
                        rhs=st_sb[slot * K:(slot + 1) * K, :],
                        start=True,
                        stop=True,
                        tile_position=(slot * K, 0),
                    )
                    nc.vector.tensor_tensor(
                        out=po_mb[:, msub * BLK:(msub + 1) * BLK],
                        in0=px_mb[:, msub * BLK:(msub + 1) * BLK],
                        in1=y_ps,
                        op=AluOpType.add,
                    )
                pmb = (2 * pp) // MB
                if (2 * pp + 1) % MB == MB - 1:
                    nc.scalar.dma_start(
                        out_d[:, pmb * MBROWS:(pmb + 1) * MBROWS], po_mb
                    )

            for pair in range(NBLOCKS // 2):
                for slot in range(2):
                    s = 2 * pair + slot
                    mb, msub = divmod(s, MB)
                    if msub == 0:
                        x_mb = xp.tile([128, MBROWS], F32)
                        nc.sync.dma_start(
                            x_mb, x_d[:, mb * MBROWS:(mb + 1) * MBROWS]
                        )
                        o_mb = op.tile([128, MBROWS], F32)
                    if slot == 0:
                        z_ps = ps_z.tile([128, BLK], F32)
                    # z[slot half] [64, 512] = T_g @ x_sub
                    nc.tensor.matmul(
                        z_ps[slot * K:(slot + 1) * K, :],
                        lhsT=ct_sb[:, grp(s) * K:(grp(s) + 1) * K],
                        rhs=x_mb[:, msub * BLK:(msub + 1) * BLK],
                        start=True,
                        stop=True,
                        tile_position=(0, slot * K),
                    )

                sig_sb = sgp.tile([128, BLK], F16)
                nc.scalar.activation(
                    sig_sb, z_ps, mybir.ActivationFunctionType.Sigmoid
                )
                # st = (z > 0) * sigmoid(z) for both packed sub-blocks at once
                st_sb = stp.tile([128, BLK], F16)
                nc.vector.scalar_tensor_tensor(
                    out=st_sb,
                    in0=z_ps,
                    scalar=0.0,
                    in1=sig_sb,
                    op0=AluOpType.is_gt,
                    op1=AluOpType.mult,
                )

                # second matmul / add / store run one pair behind so the PE
                # never waits on the current pair's sigmoid/STT
                if pend is not None:
                    flush(pend)
                pend = (st_sb, x_mb, o_mb, pair)

            flush(pend)

    nc.compile()
    return nc


def plan_permutation(s_mti):
    """Row permutation grouping keep-rows first, each group padded to a
    512-row boundary. Returns (idx_keep, idx_masked, nk)."""
    n_mask = np.ones(N, np.float32)
    n_mask[np.asarray(s_mti)] = 0.0
    t_mask = np.ones(P, np.float32)
    t_mask[T_MTI] = 0.0
    keep = (n_mask[:, None] * t_mask[None, :]).reshape(-1)   # [NP]
    keep_core = np.tile(keep, BPC)                           # [BPC*NP]
    idx_keep = np.flatnonzero(keep_core == 1.0)
    idx_masked = np.flatnonzero(keep_core == 0.0)
    nk = (len(idx_keep) + BLK - 1) // BLK
    assert nk * BLK + len(idx_masked) <= ROWS
    return idx_keep, idx_masked, nk


def host_inputs(patches, u_prompt, m_prompt, s_mti):
    patches = np.asarray(patches, dtype=np.float32)
    u = np.asarray(u_prompt, dtype=np.float32)
    m = np.asarray(m_prompt, dtype=np.float32)

    C = np.concatenate([u, m], axis=0)                       # [128, 128]
    ct = np.ascontiguousarray(C.T)                           # [D, 2K] f32
    cf = C.astype(np.float16)
    cboth = np.ascontiguousarray(
        np.concatenate([np.concatenate([cf[:K], cf[K:]], 1)] * 2, 0)
    )                                                        # [128, 256]

    idx_keep, idx_masked, nk = plan_permutation(s_mti)

    x_flat = patches.reshape(B, NP, D)
    in_maps = []
    for c in range(NCORES):
        xT = x_flat[c * BPC:(c + 1) * BPC].reshape(BPC * NP, D).T  # [D, rows]
        xs = np.zeros((D, ROWS), np.float32)
        xs[:, :len(idx_keep)] = xT[:, idx_keep]
        xs[:, nk * BLK:nk * BLK + len(idx_masked)] = xT[:, idx_masked]
        in_maps.append({"x": xs, "ct": ct, "cboth": cboth})
    return in_maps, (idx_keep, idx_masked, nk)


_NC_CACHE = {}


def kernel(patches, u_prompt, m_prompt, s_mti, s_uti=None, trace=False, **kw):
    from concourse.bass_utils import run_bass_kernel_spmd

    in_maps, (idx_keep, idx_masked, nk) = host_inputs(
        patches, u_prompt, m_prompt, s_mti
    )

    if nk not in _NC_CACHE:
        _NC_CACHE[nk] = build_nc(nk)
    nc = _NC_CACHE[nk]

    res = run_bass_kernel_spmd(nc, in_maps, list(range(NCORES)), trace=trace)
    out = np.empty((B, NP, D), np.float32)
    for c in range(NCORES):
        oT = res.results[c]["out"]                           # [D, ROWS]
        dst = out[c * BPC:(c + 1) * BPC].reshape(BPC * NP, D)
        dst[idx_keep] = oT[:, :len(idx_keep)].T
        dst[idx_masked] = oT[:, nk * BLK:nk * BLK + len(idx_masked)].T
    out = out.reshape(B, N, P, D)
    if trace:
        kernel.last_results = res
    return out


# revision 20
# speedup vs baseline: 1.0086x; 1.0086x over previous
"""Trainium2 Bass kernel for nn_AttentionPromptExtrapolation.

Reference computation (B,N,P,D,K = 32,512,25,128,64):
    keep[n,p] = (n not in s_mti) and (p != 24)            # {0,1}, same for all b
    su = sigmoid(patches @ u.T);  su *= (su>0.5) * keep
    sm = sigmoid(patches @ m.T);  sm *= (sm>0.5) * (1-keep)
    out = patches + su @ u + sm @ m

Key observation: each row (b,n,p) uses exactly ONE of the two prompt tables
(u if keep, m otherwise). The host permutes rows so all keep-rows come
first, padded to 512-row sub-block boundaries (102 sub-blocks per core).
Each sub-block is then "pure" and needs a single K=64 score matmul against
its table — no masking on the device at all:

    z  = x_block @ T.T          [64, 512]  (T = u or m by group)
    st = (z > SIG_CUT) * sigmoid(z)   (fl32 sigmoid(z) > 0.5  <=>  z > 1.5*2^-24)
    out_block = x_block + st.T @ T

Layout: the host ships patches TRANSPOSED ([D, rows] row-major) so the
contraction dim D sits on SBUF partitions with contiguous 12KB-per-partition
DMA chunks — no on-chip transposes. The whole pipeline runs in transposed
space; the host un-permutes/un-transposes the result. Two sub-blocks' scores
are packed into the two partition halves of one PSUM bank (tile_position
column offset) so sigmoid and the threshold op each run once per PAIR.
The second matmul / add / store stage runs one pair behind the score stage
so the PE never stalls waiting on sigmoid/STT.
"""

import numpy as np

import concourse.bacc as bacc
import concourse.tile as tile
from concourse import mybir
from concourse.alu_op_type import AluOpType

B, N, P, D, K = 32, 512, 25, 128, 64
K2 = 2 * K              # 128
NCORES = 8
BPC = B // NCORES       # batches per core = 4
NP = N * P              # rows per batch = 12800
BLK = 512               # rows per compute sub-block
NBLOCKS = 102           # sub-blocks per core after group padding (51 pairs)
MB = 6                  # sub-blocks per megablock DMA (17 megablocks)
MBROWS = MB * BLK       # 3072 rows = 1.5 MB per DMA
ROWS = NBLOCKS * BLK    # 52224 padded rows per core
T_MTI = 24
# largest fp32 z for which jax/XLA fl(sigmoid(z)) == 0.5 (bisected on CPU)
SIG_CUT = 8.940696716308594e-08

F32 = mybir.dt.float32
F16 = mybir.dt.float16


def build_nc(nk):
    """Build the single-core bass program. Sub-blocks [0, nk) are keep-group
    (use table u = C[0:64]); [nk, NBLOCKS) are masked-group (m = C[64:128])."""
    nc = bacc.Bacc(None, target_bir_lowering=False)

    x_d = nc.dram_tensor("x", [D, ROWS], F32, kind="ExternalInput")       # x.T
    ct_d = nc.dram_tensor("ct", [D, K2], F32, kind="ExternalInput")       # C.T
    # C fp16 replicated in both partition halves: [u | m] on parts 0:64
    # and again on parts 64:128 (mm2's contraction partitions must match
    # whichever half of st it consumes)
    cb_d = nc.dram_tensor("cboth", [K2, 2 * D], F16, kind="ExternalInput")
    out_d = nc.dram_tensor("out", [D, ROWS], F32, kind="ExternalOutput")  # out.T

    def grp(s):
        return 0 if s < nk else 1

    with tile.TileContext(nc) as tc:
        with (
            tc.tile_pool(name="consts", bufs=1) as consts,
            tc.tile_pool(name="xp", bufs=4) as xp,
            tc.tile_pool(name="sgp", bufs=3) as sgp,
            tc.tile_pool(name="stp", bufs=3) as stp,
            tc.tile_pool(name="op", bufs=3) as op,
            tc.tile_pool(name="ps_z", bufs=3, space="PSUM") as ps_z,
            tc.tile_pool(name="ps_y", bufs=4, space="PSUM") as ps_y,
        ):
            ct_sb = consts.tile([D, K2], F32)
            nc.sync.dma_start(ct_sb, ct_d[:, :])
            cb_sb = consts.tile([K2, 2 * D], F16)
            nc.sync.dma_start(cb_sb, cb_d[:, :])

            x_mb = o_mb = None
            pend = None  # previous pair's (st_sb, x_mb, o_mb, pair_idx)

            def flush(pend):
                st_sb, px_mb, po_mb, pp = pend
                for slot in range(2):
                    s = 2 * pp + slot
                    g = grp(s)
                    msub = s % MB
                    y_ps = ps_y.tile([128, BLK], F32)
                    nc.tensor.matmul(
                        y_ps,
                        lhsT=cb_sb[slot * K:(slot + 1) * K,
                                   g * D:(g + 1) * D],
                        rhs=st_sb[slot * K:(slot + 1) * K, :],
                        start=True,
                        stop=True,
                        tile_position=(slot * K, 0),
                    )
                    nc.vector.tensor_tensor(
                        out=po_mb[:, msub * BLK:(msub + 1) * BLK],
                        in0=px_mb[:, msub * BLK:(msub + 1) * BLK],
                        in1=y_ps,
                        op=AluOpType.add,
                    )
                pmb = (2 * pp) // MB
                if (2 * pp + 1) % MB == MB - 1:
                    nc.scalar.dma_start(
                        out_d[:, pmb * MBROWS:(pmb + 1) * MBROWS], po_mb
                    )

            for pair in range(NBLOCKS // 2):
                for slot in range(2):
                    s = 2 * pair + slot
                    mb, msub = divmod(s, MB)
                    if msub == 0:
                        x_mb = xp.tile([128, MBROWS], F32)
                        nc.sync.dma_start(
                            x_mb, x_d[:, mb * MBROWS:(mb + 1) * MBROWS]
                        )
                        o_mb = op.tile([128, MBROWS], F32)
                    if slot == 0:
                        z_ps = ps_z.tile([128, BLK], F32)
                    # z[slot half] [64, 512] = T_g @ x_sub
                    nc.tensor.matmul(
                        z_ps[slot * K:(slot + 1) * K, :],
                        lhsT=ct_sb[:, grp(s) * K:(grp(s) + 1) * K],
                        rhs=x_mb[:, msub * BLK:(msub + 1) * BLK],
                        start=True,
                        stop=True,
                        tile_position=(0, slot * K),
                    )

                sig_sb = sgp.tile([128, BLK], F16)
                nc.scalar.activation(
                    sig_sb, z_ps, mybir.ActivationFunctionType.Sigmoid
                )
                # st = (z > cut) * sigmoid(z), both packed sub-blocks at once
                st_sb = stp.tile([128, BLK], F16)
                nc.vector.scalar_tensor_tensor(
                    out=st_sb,
                    in0=z_ps,
                    scalar=SIG_CUT,
                    in1=sig_sb,
                    op0=AluOpType.is_gt,
                    op1=AluOpType.mult,
                )

                # second matmul / add / store run one pair behind so the PE
                # never waits on the current pair's sigmoid/STT
                if pend is not None:
                    flush(pend)
                pend = (st_sb, x_mb, o_mb, pair)

            flush(pend)

    nc.compile()
    return nc


def plan_permutation(s_mti):
    """Row permutation grouping keep-rows first, each group padded to a
    512-row boundary. Returns (idx_keep, idx_masked, nk)."""
    n_mask = np.ones(N, np.float32)
    n_mask[np.asarray(s_mti)] = 0.0
    t_mask = np.ones(P, np.float32)
    t_mask[T_MTI] = 0.0
    keep = (n_mask[:, None] * t_mask[None, :]).reshape(-1)   # [NP]
    keep_core = np.tile(keep, BPC)                           # [BPC*NP]
    idx_keep = np.flatnonzero(keep_core == 1.0)
    idx_masked = np.flatnonzero(keep_core == 0.0)
    nk = (len(idx_keep) + BLK - 1) // BLK
    assert nk * BLK + len(idx_masked) <= ROWS
    return idx_keep, idx_masked, nk


def host_inputs(patches, u_prompt, m_prompt, s_mti):
    patches = np.asarray(patches, dtype=np.float32)
    u = np.asarray(u_prompt, dtype=np.float32)
    m = np.asarray(m_prompt, dtype=np.float32)

    C = np.concatenate([u, m], axis=0)                       # [128, 128]
    ct = np.ascontiguousarray(C.T)                           # [D, 2K] f32
    cf = C.astype(np.float16)
    cboth = np.ascontiguousarray(
        np.concatenate([np.concatenate([cf[:K], cf[K:]], 1)] * 2, 0)
    )                                                        # [128, 256]

    idx_keep, idx_masked, nk = plan_permutation(s_mti)

    x_flat = patches.reshape(B, NP, D)
    in_maps = []
    for c in range(NCORES):
        xT = x_flat[c * BPC:(c + 1) * BPC].reshape(BPC * NP, D).T  # [D, rows]
        xs = np.zeros((D, ROWS), np.float32)
        xs[:, :len(idx_keep)] = xT[:, idx_keep]
        xs[:, nk * BLK:nk * BLK + len(idx_masked)] = xT[:, idx_masked]
        in_maps.append({"x": xs, "ct": ct, "cboth": cboth})
    return in_maps, (idx_keep, idx_masked, nk)


_NC_CACHE = {}


def kernel(patches, u_prompt, m_prompt, s_mti, s_uti=None, trace=False, **kw):
    from concourse.bass_utils import run_bass_kernel_spmd

    in_maps, (idx_keep, idx_masked, nk) = host_inputs(
        patches, u_prompt, m_prompt, s_mti
    )

    if nk not in _NC_CACHE:
        _NC_CACHE[nk] = build_nc(nk)
    nc = _NC_CACHE[nk]

    res = run_bass_kernel_spmd(nc, in_maps, list(range(NCORES)), trace=trace)
    out = np.empty((B, NP, D), np.float32)
    for c in range(NCORES):
        oT = res.results[c]["out"]                           # [D, ROWS]
        dst = out[c * BPC:(c + 1) * BPC].reshape(BPC * NP, D)
        dst[idx_keep] = oT[:, :len(idx_keep)].T
        dst[idx_masked] = oT[:, nk * BLK:nk * BLK + len(idx_masked)].T
    out = out.reshape(B, N, P, D)
    if trace:
        kernel.last_results = res
    return out
